# revision 36
# baseline (speedup 1.0000x reference)
"""BlurPool3D Trainium2 kernel — bf16, DMA-roofline oriented.

Depthwise 3x3x3 separable (rank-1) blur, stride 2, pad 1 on
x[2, 64, 64, 96, 96] -> y[2, 64, 32, 48, 48].

The correctness gate is rel_err < 2e-2, which admits bf16 end-to-end:
input is cast to bf16 on the host (the binomial filter's taps and tap
ratios are exact powers of two in bf16), halving HBM traffic vs fp32.
Per-core traffic 16x(64*96*96 + 32*48*48)*2B = 21.2 MB -> ~60 us DMA
roofline at ~360 GB/s effective.

Engine split (per core: 16 (n,c) pairs = 8 blocks of 2 channels;
SBUF partitions = 2 nc x 64 d):
  - DVE does the H-blur: stride-2 over h-ROWS keeps the innermost
    element step at 1, so bf16 packing engages (a stride-2 *element*
    read, as in a direct W-pass, blocks packing). scalar_tensor_tensor
    supports NO dve perf modes (measured 1 elem/cy), so the 3-tap blur
    is tensor_scalar (mid*r1, 4x) + tensor_tensor (top+bot, 2x) +
    in-place tensor_tensor add (2x), on a 97-row x tile whose row 0 is
    memset to zero (h=-1 pad). The mid*r1 of the g=0 half runs on
    ScalarE (activation scale-copy) for engine balance.
  - TensorE does W and D fused: per h-half, 3 accumulated matmuls per
    PSUM chunk, one per W tap, with a block-diagonal D-band lhsT
    [128, 64] and rhs reading the H-blurred tile at w offset (k-1)
    with element stride 2 (strides are free in the moving-operand
    AP). The w'=0 left tap (k=0) is a partial matmul over w' 1..47.
  - The two 24-row h-halves of ONE block map to PE column groups 0/1
    (tile_position (0,0)/(0,64)) writing PSUM partitions 0-63/64-127,
    so matmul+drain+output run at block granularity (short pipeline
    tail) and full 128-partition width. ScalarE drains PSUM fp32 ->
    bf16 SBUF; one 295 KB output DMA per block.
  - Input DMAs alternate between the SP and ACT HWDGE rings; the
    first block's input is split into 4 quarter-DMAs so the first
    H-blur starts ~4 us earlier.
"""

import os
import sys

for _p in ("/opt/trn_rl_repo",):
    if _p not in sys.path and os.path.isdir(_p):
        sys.path.insert(0, _p)

import ml_dtypes
import numpy as np

N, C, D, H, W = 2, 64, 64, 96, 96
DO, HO, WO = 32, 48, 48
NCORES = 8
NC_PER_CORE = (N * C) // NCORES  # 16
BLOCKS = NC_PER_CORE // 2  # 8 blocks of 2 channels each
XROWS = H + 1  # zero pad row + 96 x rows
HH = HO // 2  # 24 output rows per h-half
# h''-row chunks within a 24-row half: PSUM bank holds 512 fp32
CHUNKS = [(0, 10), (10, 10), (20, 4)]

BF16 = ml_dtypes.bfloat16

_PROGRAM_CACHE = {}


def _rank1_factors(filt):
    """Per-channel rank-1 factorization filt[c,0] = outer(d, h, w)."""
    dvec = np.empty((C, 3), np.float64)
    hvec = np.empty((C, 3), np.float64)
    wvec = np.empty((C, 3), np.float64)
    for c in range(C):
        T = filt[c, 0].astype(np.float64)
        idx = np.unravel_index(np.argmax(np.abs(T)), T.shape)
        i0, j0, k0 = idx
        piv = T[i0, j0, k0]
        if piv == 0.0:
            dvec[c] = hvec[c] = wvec[c] = 0.0
            continue
        dvec[c] = T[:, j0, k0]
        hvec[c] = T[i0, :, k0] / piv
        wvec[c] = T[i0, j0, :] / piv
        recon = np.einsum("i,j,k->ijk", dvec[c], hvec[c], wvec[c])
        resid = np.abs(recon - T).max()
        if resid > 1e-6 * max(np.abs(T).max(), 1e-30):
            raise ValueError(f"filter channel {c} is not rank-1 (resid {resid})")
    return dvec, hvec, wvec


def _build_program(uniform, r2_one):
    import concourse.bacc as bacc
    import concourse.mybir as mybir
    from concourse import tile

    dt = mybir.dt
    bf = dt.bfloat16
    add = mybir.AluOpType.add
    nc = bacc.Bacc("TRN2", target_bir_lowering=False, debug=False,
                   num_devices=NCORES)

    nbm = 1 if uniform else BLOCKS
    x = nc.dram_tensor("x", [NC_PER_CORE, D, H * W], bf,
                       kind="ExternalInput")
    bmat = nc.dram_tensor("bmat", [128, nbm * 3 * 64], bf,
                          kind="ExternalInput")
    wtaps = nc.dram_tensor("wtaps", [128, 2 * BLOCKS], dt.float32,
                           kind="ExternalInput")
    # block-native layout [b, g(h-half), ncl, d', 24*48]; host permutes
    y = nc.dram_tensor("y", [BLOCKS, 2, 2, DO, HH * WO], bf,
                       kind="ExternalOutput")

    with tile.TileContext(nc) as tc:
        with tc.tile_pool(name="const", bufs=1) as cpool, \
             tc.tile_pool(name="xp", bufs=4) as xpool, \
             tc.tile_pool(name="hp", bufs=3) as hpool, \
             tc.tile_pool(name="mp", bufs=3) as mpool, \
             tc.tile_pool(name="op", bufs=3) as opool, \
             tc.tile_pool(name="ps", bufs=8, space="PSUM") as pspool:
            bt = cpool.tile([128, nbm * 3 * 64], bf)
            wt = cpool.tile([128, 2 * BLOCKS], dt.float32)
            nc.sync.dma_start(bt[:], bmat[:])
            nc.sync.dma_start(wt[:], wtaps[:])

            def hblur(xt, hb, m, hr1, hr2, h0, hcnt, ts_act):
                """H-blur rows h' in [h0, h0+hcnt) of hb from xt.

                taps of h' are xt rows 2h', 2h'+1, 2h'+2 (xt row r = x
                row r-1; xt row 0 is the h=-1 zero pad). All slices are
                row-aligned with innermost step 1, so DVE bf16 packing
                engages (TT 2x, TS 4x). NOTE: GpSimd bulk ops are a trap
                here - they hold the shared SBUF port pair and block
                DVE's packed perf modes (exclusive lock).
                """
                rows = hb[:, h0:h0 + hcnt, :]
                mm = m[:, h0:h0 + hcnt, :]
                top = xt[:, 2 * h0 + 0:2 * h0 + 2 * hcnt - 1:2, :]
                mid = xt[:, 2 * h0 + 1:2 * h0 + 2 * hcnt:2, :]
                bot = xt[:, 2 * h0 + 2:2 * h0 + 2 * hcnt + 1:2, :]
                if ts_act:
                    nc.scalar.mul(mm, mid, hr1)
                else:
                    nc.vector.tensor_scalar_mul(mm, mid, hr1)
                if r2_one:
                    nc.vector.tensor_tensor(rows, top, bot, add)
                else:
                    nc.vector.tensor_scalar_mul(rows, bot, hr2)
                    nc.vector.tensor_tensor(rows, top, rows, add)
                nc.vector.tensor_tensor(rows, mm, rows, add)

            for b in range(BLOCKS):
                hr1 = wt[:, 2 * b:2 * b + 1]
                hr2 = wt[:, 2 * b + 1:2 * b + 2]
                src = x[2 * b:2 * b + 2].rearrange("a d f -> (a d) f")
                src = src.rearrange("p (h w) -> p h w", h=H)
                xt = xpool.tile([128, XROWS, W], bf, tag="xt")
                nc.gpsimd.memset(xt[:, 0, :], 0.0)
                hb = hpool.tile([128, HO, W], bf, tag="hb")
                m = mpool.tile([128, HO, W], bf, tag="m")
                if b == 0:
                    # fine-grained chunks + sub-split H-blur so the
                    # first DVE op starts as soon as possible
                    nc.sync.dma_start(xt[:, 1:13, :], src[:, 0:12, :])
                    nc.sync.dma_start(xt[:, 13:25, :], src[:, 12:24, :])
                    nc.sync.dma_start(xt[:, 25:37, :], src[:, 24:36, :])
                    nc.sync.dma_start(xt[:, 37:49, :], src[:, 36:48, :])
                    nc.scalar.dma_start(xt[:, 49:73, :], src[:, 48:72, :])
                    nc.scalar.dma_start(xt[:, 73:97, :], src[:, 72:96, :])
                    for h0 in (0, 6, 12, 18):
                        hblur(xt, hb, m, hr1, hr2, h0, 6, False)
                    hblur(xt, hb, m, hr1, hr2, 24, 24, False)
                elif b == BLOCKS - 1:
                    # split halves: g0 matmuls can overlap g1 H-blur in
                    # the pipeline tail
                    nc.sync.dma_start(xt[:, 1:49, :], src[:, 0:48, :])
                    nc.scalar.dma_start(xt[:, 49:97, :], src[:, 48:96, :])
                    # tail block: TS stays on DVE so the last TT2 is
                    # not coupled to the ScalarE queue
                    hblur(xt, hb, m, hr1, hr2, 0, 24, False)
                    hblur(xt, hb, m, hr1, hr2, 24, 24, False)
                else:
                    nc.sync.dma_start(xt[:, 1:49, :], src[:, 0:48, :])
                    nc.scalar.dma_start(xt[:, 49:97, :], src[:, 48:96, :])
                    # whole-block H-blur: halves merge into one slice
                    # set (fewer, larger DVE ops); TS goes to ScalarE on
                    # some blocks for engine balance
                    hblur(xt, hb, m, hr1, hr2, 0, 48, b % 3 == 1)

                # ---- fused W+D matmuls, PE column group g per h-half --
                bcol = 0 if uniform else b * 3 * 64
                pst = {}
                for h0, cnt in CHUNKS:
                    pst[h0] = pspool.tile([128, 10, WO], dt.float32,
                                          tag="ps", name="ps")
                for g in range(2):
                    for k in (1, 0, 2):
                        lhsT = bt[:, bcol + k * 64:bcol + (k + 1) * 64]
                        for h0, cnt in CHUNKS:
                            ps = pst[h0]
                            if k == 0:
                                # w = 2w'-1 exists only for w' >= 1
                                out = ps[64 * g:64 * g + 64, 0:cnt, 1:WO]
                                rhs = hb[:, HH * g + h0:HH * g + h0 + cnt,
                                         1:2 * WO - 1:2]
                            else:
                                out = ps[64 * g:64 * g + 64, 0:cnt, :]
                                rhs = hb[:, HH * g + h0:HH * g + h0 + cnt,
                                         k - 1:W:2]
                            nc.tensor.matmul(
                                out, lhsT, rhs,
                                start=(k == 1), stop=(k == 2),
                                tile_position=(0, 64 * g) if g else None)

                # out partitions (g, ncl, d'); free = 24 h'' x 48 w'
                ot = opool.tile([128, HH * WO], bf)
                ot3 = ot[:, :].rearrange("p (h w) -> p h w", h=HH)
                for h0, cnt in CHUNKS:
                    nc.scalar.copy(ot3[:, h0:h0 + cnt, :],
                                   pst[h0][:, 0:cnt, :])
                dst = y[b].rearrange("g a d f -> (g a d) f")
                # alternate rings to balance queue byte-load
                (nc.sync if b % 2 == 0 else nc.scalar).dma_start(dst, ot[:, :])
    nc.compile()
    return nc


def kernel(x, filt):
    x = np.ascontiguousarray(np.asarray(x, dtype=np.float32))
    filt = np.asarray(filt, dtype=np.float32)
    assert x.shape == (N, C, D, H, W), x.shape

    from concourse.bass_utils import run_bass_kernel_spmd

    dvec, hvec, wvec = _rank1_factors(filt)
    h0v = hvec[:, 0].copy()
    if not (np.abs(h0v) > 1e-30).all():
        raise ValueError("H-tap pivot is zero; unsupported filter")
    hr1 = hvec[:, 1] / h0v
    hr2 = hvec[:, 2] / h0v

    uniform = bool(np.all(filt == filt[:1]))
    xb = x.reshape(N * C, D, H * W).astype(BF16)

    in_maps = []
    for core in range(NCORES):
        chans = (np.arange(NC_PER_CORE) + core * NC_PER_CORE) % C
        wtp = np.empty((128, 2 * BLOCKS), np.float32)
        bm = np.zeros((128, (1 if uniform else BLOCKS) * 3 * 64), np.float32)
        for b in range(BLOCKS):
            for ncl in range(2):
                c = chans[2 * b + ncl]
                wtp[ncl * 64:(ncl + 1) * 64, 2 * b + 0] = hr1[c]
                wtp[ncl * 64:(ncl + 1) * 64, 2 * b + 1] = hr2[c]
                if uniform and b > 0:
                    continue
                # band rows (ncl*64 + d), cols (ncl*32 + d'), one band
                # per W tap k; D taps live inside the band, scaled by
                # the W tap and the H pivot.
                for k in range(3):
                    col0 = (0 if uniform else b * 3 * 64) + k * 64 + ncl * 32
                    for dp in range(DO):
                        for delta in range(3):
                            d = 2 * dp - 1 + delta
                            if 0 <= d < D:
                                bm[ncl * 64 + d, col0 + dp] = (
                                    dvec[c, delta] * wvec[c, k] * h0v[c])
        in_maps.append({
            "x": np.ascontiguousarray(
                xb[core * NC_PER_CORE:(core + 1) * NC_PER_CORE]),
            "bmat": bm.astype(BF16),
            "wtaps": wtp,
        })

    r2_one = bool(np.all(hr2 == 1.0))
    key = ("prog", uniform, r2_one)
    if key not in _PROGRAM_CACHE:
        _PROGRAM_CACHE[key] = _build_program(uniform, r2_one)
    nc = _PROGRAM_CACHE[key]

    trace = bool(int(os.environ.get("BLURPOOL_TRACE", "0")))
    kwargs = {}
    if trace and os.environ.get("BLURPOOL_TRACE_DIR"):
        kwargs["tmpdir"] = os.environ["BLURPOOL_TRACE_DIR"]
    res = run_bass_kernel_spmd(nc, in_maps, core_ids=list(range(NCORES)),
                               trace=trace, **kwargs)
    if trace:
        kernel.last_result = res

    parts = []
    for r in res.results:
        # [b, g, ncl, d', 24*48] -> [2b+ncl, d', 24g+h'', w']
        arr = np.asarray(r["y"]).reshape(BLOCKS, 2, 2, DO, HH, WO)
        arr = arr.transpose(0, 2, 3, 1, 4, 5)  # b, ncl, d', g, h'', w'
        parts.append(arr.reshape(NC_PER_CORE, DO, HO, WO))
    out = np.concatenate(parts, axis=0).astype(np.float32)
    return np.ascontiguousarray(out.reshape(N, C, DO, HO, WO))
